# revision 34
# baseline (speedup 1.0000x reference)
"""BiMamba (bidirectional Mamba-1 selective scan) on 8 Trainium2 NeuronCores.

Sharding: core c = (b, dir, half) with b = c>>2, dir = (c>>1)&1, half = c&1.
Each core computes its half of d_inner for one (batch, direction) in a
transposed [d, L] layout, fp16 matmul inputs / fp32 accumulation:
  in_proj -> depthwise conv (diagonal-weight matmuls, diagonals built
  on-device) -> silu -> x_proj partial -> pairwise AllReduce of x_dbl (f16)
  -> dt softplus -> selective scan:
       per (n, d-tile): dA = exp(A*dt) on ACT, dBu = dtu*B on DVE,
       h = tensor_tensor_scan on DVE, tmp = h*C on DVE, y += tmp on GPSIMD
  -> gate with silu(z) -> out_proj partial -> pairwise ReduceScatter of the
  out partials so each core emits a [512, L] f16 slice of d_model.
Host concatenates/transposes the slices and flips the bwd direction.
"""
import sys
sys.path.insert(0, "/opt/trn_rl_repo")
import numpy as np
from contextlib import ExitStack

import concourse.bass as bass
import concourse.mybir as mybir
import concourse.tile as tile
from concourse.vector_clock import ScopedClock

F32 = mybir.dt.float32
F16 = mybir.dt.float16
AF = mybir.ActivationFunctionType
OP = mybir.AluOpType

# ---------------------------------------------------------------- geometry
B, L, DM = 2, 2048, 1024
DI, DS, DC, DTR = 2 * DM, 16, 4, DM // 16
DH = DI // 2              # d_inner half per core
NT = DH // 128            # d-tiles per core
HALVES = 2
LC = L // HALVES          # L chunk per phase
MMT = 512                 # matmul free-dim tile
P = 128
KT = DM // P              # d_model tiles
LTN = LC // MMT

# smalls packing (columns of the [128, SMALLW] f32 tensor)
SM_CONVB = 0              # NT cols
SM_DTB = SM_CONVB + NT    # NT cols
SM_DCOL = SM_DTB + NT     # NT cols
SM_A = SM_DCOL + NT       # NT*DS cols
SM_CONVW = SM_A + NT * DS # NT*DC cols
SMALLW = SM_CONVW + NT * DC

# wcat packing (columns of the [DM, WCATW] f16 tensor): w_in | w_out | w_x
WC_WIN = 0                # 2*DH cols
WC_WOUT = WC_WIN + 2 * DH # DM cols
WC_WX = WC_WOUT + DM      # 96 cols
WCATW = WC_WX + 96

GP_EVERY = 6              # every GP_EVERY-th y-accumulate runs on DVE instead

MAXW = 1                  # codegen limit: sem waits per instruction


# ------------------------------------------------------------- tile patch
def _patched_drain_and_barrier(self, tick_clock, wait_clock):
    nop_inst = self.nc.sync.nop(nofuse=True)
    wait_clock.add_sem_waits(
        nop_inst.ins, ScopedClock({None: tick_clock.global_clock}))
    si = nop_inst.ins.sync_info
    if si is not None and si.on_wait and len(si.on_wait) > MAXW:
        extra = list(si.on_wait[MAXW:])
        del si.on_wait[MAXW:]
        for i in range(0, len(extra), MAXW):
            nop2 = self.nc.sync.nop(nofuse=True)
            nop2.ins.sync_info = mybir.SyncInfo(
                on_wait=extra[i:i + MAXW], on_update=[])
    self.nc.sync.drain()
    self.nc.all_engine_barrier()
    assert self.sems is not None
    popped = self.nc._tile_sem_poison_stack.pop()
    assert popped is self._sem_poison
    self.nc.clear_and_free_semaphores(list(self.sems.allocated().values()))
    self.nc.all_engine_barrier()


tile.TileContext._drain_and_barrier = _patched_drain_and_barrier


def split_multiwaits(nc, maxw=MAXW):
    ctr = 0
    for fn in nc.m.functions:
        for blk in fn.blocks:
            il = list(blk.instructions)
            out = []
            changed = False
            for ins in il:
                si = getattr(ins, "sync_info", None)
                waits = list(si.on_wait) if (si is not None and si.on_wait) else []
                if len(waits) > maxw:
                    changed = True
                    extra, keep = waits[:-maxw], waits[-maxw:]
                    for i in range(0, len(extra), maxw):
                        nop = mybir.InstNoOp(name=f"wsplit_{ctr}", ins=[], outs=[])
                        ctr += 1
                        nop.engine = ins.engine
                        nop.sync_info = mybir.SyncInfo(
                            on_wait=extra[i:i + maxw], on_update=[])
                        out.append(nop)
                    si.on_wait = keep
                out.append(ins)
            if changed:
                blk.instructions = out
    return ctr


# ------------------------------------------------------------ bass builder
def build_nc():
    nc = bass.Bass()

    # One packed f16 input: [xh (half of x^T) | wch (batch-half of
    # w_in|w_out|w_x) | w_dt], all flattened. Fewer buffers -> less per-call
    # marshaling through the PJRT/axon tunnel.
    XH_N = (DM // 2) * L
    WCH_N = (DM // 2) * WCATW
    WDT_N = DTR * DH
    pk_d = nc.declare_dram_parameter("pk", [XH_N + WCH_N + WDT_N], F16,
                                     isOutput=False)
    xh_d = pk_d[0:XH_N].rearrange("(r c) -> r c", c=L)
    wch_d = pk_d[XH_N:XH_N + WCH_N].rearrange("(r c) -> r c", c=WCATW)
    wdt_d = pk_d[XH_N + WCH_N:XH_N + WCH_N + WDT_N].rearrange(
        "(k c) -> k c", c=DH)
    sm_d = nc.declare_dram_parameter("smalls", [P, SMALLW], F32, isOutput=False)
    outp_d = nc.declare_dram_parameter("outp", [DM // 2, L], F16, isOutput=True)

    xg = nc.dram_tensor("xg", [DM, L], F16)
    wcat = nc.dram_tensor("wcat", [DM, WCATW], F16)
    xh_b = nc.dram_tensor("xh_b", [DM // 2, L], F16)
    wch_b = nc.dram_tensor("wch_b", [DM // 2, WCATW], F16)
    ccin = [nc.dram_tensor(f"ccin{h}", [96, LC], F16) for h in range(HALVES)]
    ccout = [nc.dram_tensor(f"ccout{h}", [96, LC], F16) for h in range(HALVES)]
    opart = [nc.dram_tensor(f"opart{h}", [DM, LC], F16) for h in range(HALVES)]
    rsout = [nc.dram_tensor(f"rsout{h}", [DM // 2, LC], F16)
             for h in range(HALVES)]
    pairs = [[0, 1], [2, 3], [4, 5], [6, 7]]
    bgrps = [[0, 4], [1, 5], [2, 6], [3, 7]]

    with tile.TileContext(nc) as tc, ExitStack() as ctx:
        pool = ctx.enter_context(tc.tile_pool(name="sb", bufs=1))
        psum = ctx.enter_context(tc.tile_pool(name="ps", bufs=6, space="PSUM"))

        # on-device dedup: pair-AllGather x, batch-AllGather big weights
        # (bounce inputs into internal DRAM first: collectives can't read IO)
        bx = nc.sync.dma_start(xh_b[:], xh_d)
        ag_x = nc.gpsimd.collective_compute(
            "AllGather", OP.bypass, replica_groups=pairs,
            ins=[xh_b[:]], outs=[xg[:]])
        tile.add_dep_helper(ag_x.ins, bx.ins, reason="ag_x after bounce")
        bw = nc.sync.dma_start(wch_b[:], wch_d)
        ag_w = nc.gpsimd.collective_compute(
            "AllGather", OP.bypass, replica_groups=bgrps,
            ins=[wch_b[:]], outs=[wcat[:]])
        tile.add_dep_helper(ag_w.ins, bw.ins, reason="ag_w after bounce")

        # resident small weights
        wx_r = pool.tile([P, NT, 96], F16, tag="wx")
        d = nc.sync.dma_start(
            wx_r[:],
            wcat[:, WC_WX:WC_WX + 96].rearrange("(kt p) m -> p kt m", p=P))
        tile.add_dep_helper(d.ins, ag_w.ins, reason="wx after ag_w")
        wdt_r = pool.tile([DTR, NT, P], F16, tag="wdt")
        nc.sync.dma_start(wdt_r[:], wdt_d.rearrange("k (mt m) -> k mt m", m=P))
        sm = pool.tile([P, SMALLW], F32, tag="sm")
        nc.sync.dma_start(sm[:], sm_d[:])

        # depthwise-conv diagonal weights, built on device:
        # dmask = I (f16), cdiag[nt][:, k, :] = dmask * conv_w[:, nt*DC+k]
        dmask = pool.tile([P, P], F16, tag="dmask")
        nc.gpsimd.memset(dmask[:], 1.0)
        nc.gpsimd.affine_select(
            out=dmask[:], in_=dmask[:], compare_op=OP.is_equal, fill=0.0,
            base=0, pattern=[[-1, P]], channel_multiplier=1)
        cdiag = []
        for nt in range(NT):
            cd = pool.tile([P, DC, P], F16, tag=f"cd{nt}", name=f"cd{nt}")
            for k in range(DC):
                nc.vector.tensor_scalar_mul(
                    cd[:, k, :], dmask[:],
                    sm[:, SM_CONVW + nt * DC + k:SM_CONVW + nt * DC + k + 1])
            cdiag.append(cd)

        halo = [pool.tile([P, DC - 1], F16, tag=f"halo{nt}", name=f"halo{nt}")
                for nt in range(NT)]
        states = pool.tile([P, DS * NT], F32, tag="states")

        xt_re = xg[:].rearrange("(kt p) l -> p kt l", p=P)
        st = [dict() for _ in range(HALVES)]

        def s123(half):
            """in_proj -> conv/silu -> x_proj partial -> start AllReduce."""
            l0 = half * LC
            xt_t = []
            for kt in range(KT):
                t = pool.tile([P, LC], F16, tag="big", bufs=8)
                d = nc.sync.dma_start(t[:], xt_re[:, kt, l0:l0 + LC])
                tile.add_dep_helper(d.ins, ag_x.ins, reason="xt after ag_x")
                xt_t.append(t)
            xi_t = []
            sz_t = []
            for mt in range(2 * NT):
                win_t = pool.tile([P, KT, P], F16, tag="win", bufs=2)
                d = nc.sync.dma_start(
                    win_t[:],
                    wcat[:, WC_WIN + mt * P:WC_WIN + (mt + 1) * P].rearrange(
                        "(kt p) q -> p kt q", p=P))
                tile.add_dep_helper(d.ins, ag_w.ins, reason="win after ag_w")
                if mt < NT:
                    xi = pool.tile([P, DC - 1 + LC], F16, tag="xi", bufs=8)
                    xi_t.append(xi)
                else:
                    sz = pool.tile([P, LC], F16, tag=f"sz{half}", bufs=8)
                    sz_t.append(sz)
                for lt in range(LTN):
                    acc = psum.tile([P, MMT], F32, tag="mm")
                    for kt in range(KT):
                        nc.tensor.matmul(
                            acc[:], win_t[:, kt, :],
                            xt_t[kt][:, lt * MMT:(lt + 1) * MMT],
                            start=(kt == 0), stop=(kt == KT - 1))
                    if mt < NT:
                        nc.scalar.copy(
                            xi_t[mt][:, DC - 1 + lt * MMT:DC - 1 + (lt + 1) * MMT],
                            acc[:])
                    else:
                        nc.scalar.activation(
                            sz_t[mt - NT][:, lt * MMT:(lt + 1) * MMT],
                            acc[:], AF.Silu)

            # depthwise conv + bias + silu -> u
            u_t = []
            for nt in range(NT):
                if half == 0:
                    nc.gpsimd.memset(halo[nt][:], 0.0)
                nc.vector.tensor_copy(xi_t[nt][:, 0:DC - 1], halo[nt][:])
                u = pool.tile([P, LC], F16, tag="xi", bufs=8)
                for lt in range(LTN):
                    acc = psum.tile([P, MMT], F32, tag="mm")
                    for k in range(DC):
                        nc.tensor.matmul(
                            acc[:], cdiag[nt][:, k, :],
                            xi_t[nt][:, lt * MMT + k:lt * MMT + k + MMT],
                            start=(k == 0), stop=(k == DC - 1))
                    nc.scalar.activation(
                        u[:, lt * MMT:(lt + 1) * MMT], acc[:], AF.Silu,
                        bias=sm[:, SM_CONVB + nt:SM_CONVB + nt + 1])
                # save halo for next half (before xi slot recycles)
                nc.vector.tensor_copy(
                    halo[nt][:], xi_t[nt][:, LC:LC + DC - 1])
                u_t.append(u)

            # x_proj partial [96, LC] -> pairwise AllReduce (async)
            xdblp = pool.tile([96, LC], F16, tag="xdblp", bufs=2)
            for lt in range(LTN):
                acc96 = psum.tile([96, MMT], F32, tag="mm96", bufs=2)
                for nt in range(NT):
                    nc.tensor.matmul(
                        acc96[:], wx_r[:, nt, :],
                        u_t[nt][:, lt * MMT:(lt + 1) * MMT],
                        start=(nt == 0), stop=(nt == NT - 1))
                nc.scalar.copy(xdblp[:, lt * MMT:(lt + 1) * MMT], acc96[:])
            dma_in = nc.sync.dma_start(ccin[half][:], xdblp[:])
            cc = nc.gpsimd.collective_compute(
                "AllReduce", OP.add, replica_groups=pairs,
                ins=[ccin[half][:]], outs=[ccout[half][:]])
            tile.add_dep_helper(cc.ins, dma_in.ins, reason="cc after dma_in")
            st[half].update(sz_t=sz_t, u_t=u_t, cc=cc)

        def s5(half):
            """dt = softplus(Wdt@dtr + b); dtu = dt*u; y = D*u."""
            u_t = st[half]["u_t"]
            xdbl = pool.tile([96, LC], F16, tag="xdbl", bufs=2)
            dma_out = nc.sync.dma_start(xdbl[:], ccout[half][:])
            tile.add_dep_helper(dma_out.ins, st[half]["cc"].ins,
                                reason="read after cc")
            dt_t, dtu_t, y_t = [], [], []
            for nt in range(NT):
                dt = pool.tile([P, LC], F16, tag="dt", bufs=8)
                for lt in range(LTN):
                    acc = psum.tile([P, MMT], F32, tag="mm")
                    nc.tensor.matmul(
                        acc[:], wdt_r[:, nt, :],
                        xdbl[0:DTR, lt * MMT:(lt + 1) * MMT],
                        start=True, stop=True)
                    e = pool.tile([P, MMT], F32, tag="spe", bufs=2)
                    nc.scalar.activation(e[:], acc[:], AF.Exp,
                                         bias=sm[:, SM_DTB + nt:SM_DTB + nt + 1])
                    nc.scalar.activation(
                        dt[:, lt * MMT:(lt + 1) * MMT], e[:], AF.Ln, bias=1.0)
                dt_t.append(dt)
                dtu = pool.tile([P, LC], F16, tag="dtu", bufs=8)
                nc.vector.tensor_tensor(dtu[:], dt[:], u_t[nt][:], OP.mult)
                dtu_t.append(dtu)
                y = pool.tile([P, LC], F16, tag="y", bufs=8)
                nc.scalar.mul(y[:], u_t[nt][:],
                              sm[:, SM_DCOL + nt:SM_DCOL + nt + 1])  # y = D*u
                y_t.append(y)
            st[half].update(dt_t=dt_t, dtu_t=dtu_t, y_t=y_t)

        def s6(half):
            """Selective scan. Per state n: pass A computes dA (ACT) + dBu
            (DVE) + scan (Pool, software-pipelined); pass B h*C + y+= (DVE)."""
            dt_t, dtu_t, y_t = (st[half][k] for k in ("dt_t", "dtu_t", "y_t"))
            for n in range(DS):
                Bb = pool.tile([P, LC], F16, tag="bc", bufs=4)
                nc.sync.dma_start(
                    Bb[:], ccout[half][DTR + n:DTR + n + 1, :]
                    .partition_broadcast(P))
                Cb = pool.tile([P, LC], F16, tag="bc", bufs=4)
                nc.sync.dma_start(
                    Cb[:], ccout[half][DTR + DS + n:DTR + DS + n + 1, :]
                    .partition_broadcast(P))
                h_t = []
                for nt in range(NT):
                    dA = pool.tile([P, LC], F32, tag="dA", bufs=2)
                    nc.scalar.activation(
                        dA[:], dt_t[nt][:], AF.Exp,
                        scale=sm[:, SM_A + nt * DS + n:SM_A + nt * DS + n + 1])
                    dBu = pool.tile([P, LC], F16, tag="dbu", bufs=6)
                    nc.vector.tensor_tensor(dBu[:], dtu_t[nt][:], Bb[:],
                                            OP.mult)
                    init = 0.0 if half == 0 else states[:, n * NT + nt:
                                                        n * NT + nt + 1]
                    h = pool.tile([P, LC], F16, tag="h16", bufs=4)
                    nc.vector.tensor_tensor_scan(
                        h[:], dA[:], dBu[:], init, OP.mult, OP.add)
                    h_t.append(h)
                for nt in range(NT):
                    h = h_t[nt]
                    if half < HALVES - 1:
                        nc.scalar.copy(
                            states[:, n * NT + nt:n * NT + nt + 1],
                            h[:, LC - 1:LC])
                    tmp = pool.tile([P, LC], F16, tag="tmp16", bufs=3)
                    nc.vector.tensor_tensor(tmp[:], h[:], Cb[:], OP.mult)
                    nc.gpsimd.tensor_tensor(y_t[nt][:], y_t[nt][:],
                                            tmp[:], OP.add)

        def s78(half):
            """Gate + out_proj partial -> pairwise ReduceScatter -> output."""
            l0 = half * LC
            y_t, sz_t = st[half]["y_t"], st[half]["sz_t"]
            yg_t = []
            for nt in range(NT):
                yg = pool.tile([P, LC], F16, tag="yg", bufs=8)
                nc.vector.tensor_tensor(yg[:], y_t[nt][:], sz_t[nt][:], OP.mult)
                yg_t.append(yg)
            out_dmas = []
            for mt in range(KT):
                wout_t = pool.tile([P, NT, P], F16, tag="wout", bufs=2)
                d = nc.sync.dma_start(
                    wout_t[:],
                    wcat[:, WC_WOUT + mt * P:WC_WOUT + (mt + 1) * P].rearrange(
                        "(kt p) q -> p kt q", p=P))
                tile.add_dep_helper(d.ins, ag_w.ins, reason="wout after ag_w")
                for lt in range(LTN):
                    acc = psum.tile([P, MMT], F32, tag="mm")
                    for kt in range(NT):
                        nc.tensor.matmul(
                            acc[:], wout_t[:, kt, :],
                            yg_t[kt][:, lt * MMT:(lt + 1) * MMT],
                            start=(kt == 0), stop=(kt == NT - 1))
                    o = pool.tile([P, MMT], F16, tag="op", bufs=2)
                    nc.scalar.copy(o[:], acc[:])
                    d = nc.sync.dma_start(
                        opart[half][mt * P:(mt + 1) * P,
                                    lt * MMT:(lt + 1) * MMT], o[:])
                    out_dmas.append(d)
            rs = nc.gpsimd.collective_compute(
                "ReduceScatter", OP.add, replica_groups=pairs,
                ins=[opart[half][:]], outs=[rsout[half][:]])
            for d in out_dmas:
                tile.add_dep_helper(rs.ins, d.ins, reason="rs after out dma")
            dcp = nc.sync.dma_start(outp_d[:, l0:l0 + LC], rsout[half][:])
            tile.add_dep_helper(dcp.ins, rs.ins, reason="out copy after rs")

        # Interleaved phase order: half-1's matmul-heavy front end (s123) is
        # issued before half-0's scan so PE/ACT work overlaps DVE/Pool work.
        s123(0)
        s5(0)
        s123(1)
        s6(0)
        s78(0)
        s5(1)
        s6(1)
        s78(1)

    split_multiwaits(nc)
    return nc


# ------------------------------------------------------------- host side
def _prep_core_inputs(inputs, b, dir_, half):
    pre = "f_" if dir_ == 0 else "b_"
    x = np.asarray(inputs["x"][b], dtype=np.float32)          # [L, DM]
    if dir_ == 1:
        x = x[::-1]
    sl = slice(half * DH, (half + 1) * DH)

    w_in_full = np.asarray(inputs[pre + "in_proj_w"], np.float32)  # [2DI, DM]
    w_in = np.concatenate([w_in_full[sl], w_in_full[DI + half * DH:
                                                    DI + (half + 1) * DH]], 0)
    conv_w = np.asarray(inputs[pre + "conv_w"], np.float32)[sl, 0]  # [DH, DC]
    conv_b = np.asarray(inputs[pre + "conv_b"], np.float32)[sl]
    w_x = np.asarray(inputs[pre + "x_proj_w"], np.float32)[:, sl]   # [96, DH]
    w_dt = np.asarray(inputs[pre + "dt_proj_w"], np.float32)[sl]    # [DH, DTR]
    dt_b = np.asarray(inputs[pre + "dt_proj_b"], np.float32)[sl]
    A = -np.exp(np.asarray(inputs[pre + "A_log"], np.float32))[sl]  # [DH, DS]
    Dp = np.asarray(inputs[pre + "D"], np.float32)[sl]
    w_out = np.asarray(inputs[pre + "out_proj_w"], np.float32)[:, sl]  # [DM,DH]

    smalls = np.zeros((P, SMALLW), np.float32)
    smalls[:, SM_CONVB:SM_CONVB + NT] = conv_b.reshape(NT, P).T
    smalls[:, SM_DTB:SM_DTB + NT] = dt_b.reshape(NT, P).T
    smalls[:, SM_DCOL:SM_DCOL + NT] = Dp.reshape(NT, P).T
    smalls[:, SM_A:SM_A + NT * DS] = (
        A.reshape(NT, P, DS).transpose(1, 0, 2).reshape(P, NT * DS))
    smalls[:, SM_CONVW:SM_CONVW + NT * DC] = (
        conv_w.reshape(NT, P, DC).transpose(1, 0, 2).reshape(P, NT * DC))

    # wcat = [w_in.T | w_out.T | w_x.T] as [DM, WCATW]; core ships its
    # batch-half of the rows (AllGathered with core c^4 on device).
    wcat = np.concatenate([w_in.T, w_out.T, w_x.T], axis=1)   # [DM, WCATW]
    xt = x.T                                                  # [DM, L]
    pk = np.concatenate([
        np.ascontiguousarray(
            xt[half * (DM // 2):(half + 1) * (DM // 2)]).astype(
                np.float16).ravel(),
        np.ascontiguousarray(
            wcat[b * (DM // 2):(b + 1) * (DM // 2)]).astype(
                np.float16).ravel(),
        np.ascontiguousarray(w_dt.T).astype(np.float16).ravel(),
    ])
    return {"pk": pk, "smalls": smalls}


_CACHE = {}


def _get_nc():
    if "nc" not in _CACHE:
        _CACHE["nc"] = build_nc()
    return _CACHE["nc"]


def _make_runner():
    """Jitted 8-core PJRT runner. Outputs get fresh device buffers (no
    donated zero inputs — the kernel writes every output element)."""
    import jax
    from jax.sharding import Mesh, PartitionSpec
    from jax.experimental.shard_map import shard_map
    from concourse import bass2jax
    from concourse.bass2jax import _bass_exec_p, install_neuronx_cc_hook

    install_neuronx_cc_hook()
    nc = _get_nc()
    pname = nc.partition_id_tensor.name if nc.partition_id_tensor else None
    in_names, out_names, out_avals = [], [], []
    for alloc in nc.m.functions[0].allocations:
        if not isinstance(alloc, mybir.MemoryLocationSet):
            continue
        name = alloc.memorylocations[0].name
        if alloc.kind == "ExternalInput":
            if name != pname:
                in_names.append(name)
        elif alloc.kind == "ExternalOutput":
            out_names.append(name)
            out_avals.append(jax.core.ShapedArray(
                tuple(alloc.tensor_shape), mybir.dt.np(alloc.dtype)))
    all_names = in_names
    if pname is not None:
        all_names = all_names + [pname]

    def _body(*args):
        operands = list(args)
        if pname is not None:
            operands.append(bass2jax.partition_id_tensor())
        outs = _bass_exec_p.bind(
            *operands, out_avals=tuple(out_avals), in_names=tuple(all_names),
            out_names=tuple(out_names), lowering_input_output_aliases=(),
            sim_require_finite=False, sim_require_nnan=False, nc=nc)
        return tuple(outs)

    devices = jax.devices()[:8]
    mesh = Mesh(np.asarray(devices), ("core",))
    nin = len(in_names)
    fn = jax.jit(shard_map(
        _body, mesh=mesh, in_specs=(PartitionSpec("core"),) * nin,
        out_specs=(PartitionSpec("core"),) * len(out_names), check_rep=False),
        keep_unused=True)
    return fn, in_names, out_names, out_avals


def _get_runner():
    if "runner" not in _CACHE:
        _CACHE["runner"] = _make_runner()
    return _CACHE["runner"]


def _concat_inputs(in_maps):
    import jax
    fn, in_names, out_names, out_avals = _get_runner()
    concat = [np.concatenate([np.asarray(m[k]) for m in in_maps], axis=0)
              for k in in_names]
    return [jax.device_put(a) for a in concat]


def _run(in_maps):
    import jax
    fn, in_names, out_names, out_avals = _get_runner()
    args = _concat_inputs(in_maps)
    outs = [np.asarray(o) for o in fn(*args)]
    return [
        {k: outs[i].reshape(8, *out_avals[i].shape)[c]
         for i, k in enumerate(out_names)}
        for c in range(8)
    ]


def run_timed(in_maps, iters=5):
    """Steady-state per-invocation time: issue a batch of executions
    back-to-back (the runtime pipelines host->device transfer with
    execution), block once, divide. Min over rounds."""
    import time as _t
    import jax
    fn, *_ = _get_runner()
    args = _concat_inputs(in_maps)
    jax.block_until_ready(fn(*args))
    batch = max(iters, 256)
    best = float("inf")
    for _ in range(3):
        t0 = _t.perf_counter()
        o = None
        for _ in range(batch):
            o = fn(*args)
        jax.block_until_ready(o)
        best = min(best, (_t.perf_counter() - t0) / batch)
    return best


def make_in_maps(inputs):
    return [
        _prep_core_inputs(inputs, c >> 2, (c >> 1) & 1, c & 1)
        for c in range(8)
    ]


def kernel(**inputs):
    in_maps = make_in_maps(inputs)
    res = _run(in_maps)
    out = np.zeros((B, L, 2 * DM), np.float32)
    for b in range(B):
        for dir_ in range(2):
            for half in range(2):
                c = (b << 2) | (dir_ << 1) | half
                part = res[c]["outp"].astype(np.float32)      # [DM/2, L]
                if dir_ == 1:
                    part = part[:, ::-1]
                col0 = dir_ * DM + half * (DM // 2)
                out[b, :, col0:col0 + DM // 2] = part.T
    return out


# revision 35
# speedup vs baseline: 1.0265x; 1.0265x over previous
"""BiMamba (bidirectional Mamba-1 selective scan) on 8 Trainium2 NeuronCores.

Sharding: core c = (b, dir, half) with b = c>>2, dir = (c>>1)&1, half = c&1.
Each core computes its half of d_inner for one (batch, direction) in a
transposed [d, L] layout, fp16 matmul inputs / fp32 accumulation:
  in_proj -> depthwise conv (diagonal-weight matmuls, diagonals built
  on-device) -> silu -> x_proj partial -> pairwise AllReduce of x_dbl (f16)
  -> dt softplus -> selective scan:
       per (n, d-tile): dA = exp(A*dt) on ACT, dBu = dtu*B on DVE,
       h = tensor_tensor_scan on DVE, tmp = h*C on DVE, y += tmp on GPSIMD
  -> gate with silu(z) -> out_proj partial -> pairwise ReduceScatter of the
  out partials so each core emits a [512, L] f16 slice of d_model.
Host concatenates/transposes the slices and flips the bwd direction.
"""
import sys
sys.path.insert(0, "/opt/trn_rl_repo")
import numpy as np
from contextlib import ExitStack

import concourse.bass as bass
import concourse.mybir as mybir
import concourse.tile as tile
from concourse.vector_clock import ScopedClock

F32 = mybir.dt.float32
F16 = mybir.dt.float16
AF = mybir.ActivationFunctionType
OP = mybir.AluOpType

# ---------------------------------------------------------------- geometry
B, L, DM = 2, 2048, 1024
DI, DS, DC, DTR = 2 * DM, 16, 4, DM // 16
DH = DI // 2              # d_inner half per core
NT = DH // 128            # d-tiles per core
HALVES = 2
LC = L // HALVES          # L chunk per phase
MMT = 512                 # matmul free-dim tile
P = 128
KT = DM // P              # d_model tiles
LTN = LC // MMT

# smalls packing (columns of the [128, SMALLW] f32 tensor)
SM_CONVB = 0              # NT cols
SM_DTB = SM_CONVB + NT    # NT cols
SM_DCOL = SM_DTB + NT     # NT cols
SM_A = SM_DCOL + NT       # NT*DS cols
SM_CONVW = SM_A + NT * DS # NT*DC cols
SMALLW = SM_CONVW + NT * DC

# wcat packing (columns of the [DM, WCATW] f16 tensor): w_in | w_out | w_x
WC_WIN = 0                # 2*DH cols
WC_WOUT = WC_WIN + 2 * DH # DM cols
WC_WX = WC_WOUT + DM      # 96 cols
WCATW = WC_WX + 96

GP_EVERY = 6              # every GP_EVERY-th y-accumulate runs on DVE instead

MAXW = 1                  # codegen limit: sem waits per instruction


# ------------------------------------------------------------- tile patch
def _patched_drain_and_barrier(self, tick_clock, wait_clock):
    nop_inst = self.nc.sync.nop(nofuse=True)
    wait_clock.add_sem_waits(
        nop_inst.ins, ScopedClock({None: tick_clock.global_clock}))
    si = nop_inst.ins.sync_info
    if si is not None and si.on_wait and len(si.on_wait) > MAXW:
        extra = list(si.on_wait[MAXW:])
        del si.on_wait[MAXW:]
        for i in range(0, len(extra), MAXW):
            nop2 = self.nc.sync.nop(nofuse=True)
            nop2.ins.sync_info = mybir.SyncInfo(
                on_wait=extra[i:i + MAXW], on_update=[])
    self.nc.sync.drain()
    self.nc.all_engine_barrier()
    assert self.sems is not None
    popped = self.nc._tile_sem_poison_stack.pop()
    assert popped is self._sem_poison
    self.nc.clear_and_free_semaphores(list(self.sems.allocated().values()))
    self.nc.all_engine_barrier()


tile.TileContext._drain_and_barrier = _patched_drain_and_barrier


def split_multiwaits(nc, maxw=MAXW):
    ctr = 0
    for fn in nc.m.functions:
        for blk in fn.blocks:
            il = list(blk.instructions)
            out = []
            changed = False
            for ins in il:
                si = getattr(ins, "sync_info", None)
                waits = list(si.on_wait) if (si is not None and si.on_wait) else []
                if len(waits) > maxw:
                    changed = True
                    extra, keep = waits[:-maxw], waits[-maxw:]
                    for i in range(0, len(extra), maxw):
                        nop = mybir.InstNoOp(name=f"wsplit_{ctr}", ins=[], outs=[])
                        ctr += 1
                        nop.engine = ins.engine
                        nop.sync_info = mybir.SyncInfo(
                            on_wait=extra[i:i + maxw], on_update=[])
                        out.append(nop)
                    si.on_wait = keep
                out.append(ins)
            if changed:
                blk.instructions = out
    return ctr


# ------------------------------------------------------------ bass builder
def build_nc():
    nc = bass.Bass()

    # One packed f16 input: [xh (half of x^T) | wch (batch-half of
    # w_in|w_out|w_x) | w_dt], all flattened. Fewer buffers -> less per-call
    # marshaling through the PJRT/axon tunnel.
    XH_N = (DM // 2) * L
    WCH_N = (DM // 2) * WCATW
    WDT_N = DTR * DH
    pk_d = nc.declare_dram_parameter("pk", [XH_N + WCH_N + WDT_N], F16,
                                     isOutput=False)
    xh_d = pk_d[0:XH_N].rearrange("(r c) -> r c", c=L)
    wch_d = pk_d[XH_N:XH_N + WCH_N].rearrange("(r c) -> r c", c=WCATW)
    wdt_d = pk_d[XH_N + WCH_N:XH_N + WCH_N + WDT_N].rearrange(
        "(k c) -> k c", c=DH)
    sm_d = nc.declare_dram_parameter("smalls", [P, SMALLW], F32, isOutput=False)
    outp_d = nc.declare_dram_parameter("outp", [DM // 2, L], F16, isOutput=True)

    xg = nc.dram_tensor("xg", [DM, L], F16)
    wcat = nc.dram_tensor("wcat", [DM, WCATW], F16)
    xh_b = nc.dram_tensor("xh_b", [DM // 2, L], F16)
    wch_b = nc.dram_tensor("wch_b", [DM // 2, WCATW], F16)
    ccin = [nc.dram_tensor(f"ccin{h}", [96, LC], F16) for h in range(HALVES)]
    ccout = [nc.dram_tensor(f"ccout{h}", [96, LC], F16) for h in range(HALVES)]
    opart = [nc.dram_tensor(f"opart{h}", [DM, LC], F16) for h in range(HALVES)]
    rsout = [nc.dram_tensor(f"rsout{h}", [DM // 2, LC], F16)
             for h in range(HALVES)]
    pairs = [[0, 1], [2, 3], [4, 5], [6, 7]]
    bgrps = [[0, 4], [1, 5], [2, 6], [3, 7]]

    with tile.TileContext(nc) as tc, ExitStack() as ctx:
        pool = ctx.enter_context(tc.tile_pool(name="sb", bufs=1))
        psum = ctx.enter_context(tc.tile_pool(name="ps", bufs=6, space="PSUM"))

        # on-device dedup: pair-AllGather x, batch-AllGather big weights
        # (bounce inputs into internal DRAM first: collectives can't read IO)
        bx = nc.sync.dma_start(xh_b[:], xh_d)
        ag_x = nc.gpsimd.collective_compute(
            "AllGather", OP.bypass, replica_groups=pairs,
            ins=[xh_b[:]], outs=[xg[:]])
        tile.add_dep_helper(ag_x.ins, bx.ins, reason="ag_x after bounce")
        bw = nc.sync.dma_start(wch_b[:], wch_d)
        ag_w = nc.gpsimd.collective_compute(
            "AllGather", OP.bypass, replica_groups=bgrps,
            ins=[wch_b[:]], outs=[wcat[:]])
        tile.add_dep_helper(ag_w.ins, bw.ins, reason="ag_w after bounce")

        # resident small weights
        wx_r = pool.tile([P, NT, 96], F16, tag="wx")
        d = nc.sync.dma_start(
            wx_r[:],
            wcat[:, WC_WX:WC_WX + 96].rearrange("(kt p) m -> p kt m", p=P))
        tile.add_dep_helper(d.ins, ag_w.ins, reason="wx after ag_w")
        wdt_r = pool.tile([DTR, NT, P], F16, tag="wdt")
        nc.sync.dma_start(wdt_r[:], wdt_d.rearrange("k (mt m) -> k mt m", m=P))
        sm = pool.tile([P, SMALLW], F32, tag="sm")
        nc.sync.dma_start(sm[:], sm_d[:])

        # depthwise-conv diagonal weights, built on device:
        # dmask = I (f16), cdiag[nt][:, k, :] = dmask * conv_w[:, nt*DC+k]
        dmask = pool.tile([P, P], F16, tag="dmask")
        nc.gpsimd.memset(dmask[:], 1.0)
        nc.gpsimd.affine_select(
            out=dmask[:], in_=dmask[:], compare_op=OP.is_equal, fill=0.0,
            base=0, pattern=[[-1, P]], channel_multiplier=1)
        cdiag = []
        for nt in range(NT):
            cd = pool.tile([P, DC, P], F16, tag=f"cd{nt}", name=f"cd{nt}")
            for k in range(DC):
                nc.vector.tensor_scalar_mul(
                    cd[:, k, :], dmask[:],
                    sm[:, SM_CONVW + nt * DC + k:SM_CONVW + nt * DC + k + 1])
            cdiag.append(cd)

        halo = [pool.tile([P, DC - 1], F16, tag=f"halo{nt}", name=f"halo{nt}")
                for nt in range(NT)]
        states = pool.tile([P, DS * NT], F32, tag="states")

        xt_re = xg[:].rearrange("(kt p) l -> p kt l", p=P)
        st = [dict() for _ in range(HALVES)]

        def s123(half):
            """in_proj -> conv/silu -> x_proj partial -> start AllReduce."""
            l0 = half * LC
            xt_t = []
            for kt in range(KT):
                t = pool.tile([P, LC], F16, tag="big", bufs=8)
                d = nc.sync.dma_start(t[:], xt_re[:, kt, l0:l0 + LC])
                tile.add_dep_helper(d.ins, ag_x.ins, reason="xt after ag_x")
                xt_t.append(t)
            xi_t = []
            sz_t = []
            for mt in range(2 * NT):
                win_t = pool.tile([P, KT, P], F16, tag="win", bufs=2)
                d = nc.sync.dma_start(
                    win_t[:],
                    wcat[:, WC_WIN + mt * P:WC_WIN + (mt + 1) * P].rearrange(
                        "(kt p) q -> p kt q", p=P))
                tile.add_dep_helper(d.ins, ag_w.ins, reason="win after ag_w")
                if mt < NT:
                    xi = pool.tile([P, DC - 1 + LC], F16, tag="xi", bufs=8)
                    xi_t.append(xi)
                else:
                    sz = pool.tile([P, LC], F16, tag=f"sz{half}", bufs=8)
                    sz_t.append(sz)
                for lt in range(LTN):
                    acc = psum.tile([P, MMT], F32, tag="mm")
                    for kt in range(KT):
                        nc.tensor.matmul(
                            acc[:], win_t[:, kt, :],
                            xt_t[kt][:, lt * MMT:(lt + 1) * MMT],
                            start=(kt == 0), stop=(kt == KT - 1))
                    if mt < NT:
                        nc.scalar.copy(
                            xi_t[mt][:, DC - 1 + lt * MMT:DC - 1 + (lt + 1) * MMT],
                            acc[:])
                    else:
                        nc.scalar.activation(
                            sz_t[mt - NT][:, lt * MMT:(lt + 1) * MMT],
                            acc[:], AF.Silu)

            # depthwise conv + bias + silu -> u
            u_t = []
            for nt in range(NT):
                if half == 0:
                    nc.gpsimd.memset(halo[nt][:], 0.0)
                nc.vector.tensor_copy(xi_t[nt][:, 0:DC - 1], halo[nt][:])
                u = pool.tile([P, LC], F16, tag="xi", bufs=8)
                for lt in range(LTN):
                    acc = psum.tile([P, MMT], F32, tag="mm")
                    for k in range(DC):
                        nc.tensor.matmul(
                            acc[:], cdiag[nt][:, k, :],
                            xi_t[nt][:, lt * MMT + k:lt * MMT + k + MMT],
                            start=(k == 0), stop=(k == DC - 1))
                    nc.scalar.activation(
                        u[:, lt * MMT:(lt + 1) * MMT], acc[:], AF.Silu,
                        bias=sm[:, SM_CONVB + nt:SM_CONVB + nt + 1])
                # save halo for next half (before xi slot recycles)
                nc.vector.tensor_copy(
                    halo[nt][:], xi_t[nt][:, LC:LC + DC - 1])
                u_t.append(u)

            # x_proj partial [96, LC] -> pairwise AllReduce (async)
            xdblp = pool.tile([96, LC], F16, tag="xdblp", bufs=2)
            for lt in range(LTN):
                acc96 = psum.tile([96, MMT], F32, tag="mm96", bufs=2)
                for nt in range(NT):
                    nc.tensor.matmul(
                        acc96[:], wx_r[:, nt, :],
                        u_t[nt][:, lt * MMT:(lt + 1) * MMT],
                        start=(nt == 0), stop=(nt == NT - 1))
                nc.scalar.copy(xdblp[:, lt * MMT:(lt + 1) * MMT], acc96[:])
            dma_in = nc.sync.dma_start(ccin[half][:], xdblp[:])
            cc = nc.gpsimd.collective_compute(
                "AllReduce", OP.add, replica_groups=pairs,
                ins=[ccin[half][:]], outs=[ccout[half][:]])
            tile.add_dep_helper(cc.ins, dma_in.ins, reason="cc after dma_in")
            st[half].update(sz_t=sz_t, u_t=u_t, cc=cc)

        def s5(half):
            """dt = softplus(Wdt@dtr + b); dtu = dt*u; y = D*u."""
            u_t = st[half]["u_t"]
            xdbl = pool.tile([96, LC], F16, tag="xdbl", bufs=2)
            dma_out = nc.sync.dma_start(xdbl[:], ccout[half][:])
            tile.add_dep_helper(dma_out.ins, st[half]["cc"].ins,
                                reason="read after cc")
            dt_t, dtu_t, y_t = [], [], []
            for nt in range(NT):
                dt = pool.tile([P, LC], F16, tag="dt", bufs=8)
                for lt in range(LTN):
                    acc = psum.tile([P, MMT], F32, tag="mm")
                    nc.tensor.matmul(
                        acc[:], wdt_r[:, nt, :],
                        xdbl[0:DTR, lt * MMT:(lt + 1) * MMT],
                        start=True, stop=True)
                    e = pool.tile([P, MMT], F32, tag="spe", bufs=2)
                    nc.scalar.activation(e[:], acc[:], AF.Exp,
                                         bias=sm[:, SM_DTB + nt:SM_DTB + nt + 1])
                    nc.scalar.activation(
                        dt[:, lt * MMT:(lt + 1) * MMT], e[:], AF.Ln, bias=1.0)
                dt_t.append(dt)
                dtu = pool.tile([P, LC], F16, tag="dtu", bufs=8)
                nc.vector.tensor_tensor(dtu[:], dt[:], u_t[nt][:], OP.mult)
                dtu_t.append(dtu)
                y = pool.tile([P, LC], F16, tag="y", bufs=8)
                nc.scalar.mul(y[:], u_t[nt][:],
                              sm[:, SM_DCOL + nt:SM_DCOL + nt + 1])  # y = D*u
                y_t.append(y)
            st[half].update(dt_t=dt_t, dtu_t=dtu_t, y_t=y_t)

        def s6(half):
            """Selective scan. Per state n: pass A computes dA (ACT) + dBu
            (DVE) + scan (Pool, software-pipelined); pass B h*C + y+= (DVE)."""
            dt_t, dtu_t, y_t = (st[half][k] for k in ("dt_t", "dtu_t", "y_t"))
            for n in range(DS):
                Bb = pool.tile([P, LC], F16, tag="bc", bufs=4)
                nc.sync.dma_start(
                    Bb[:], ccout[half][DTR + n:DTR + n + 1, :]
                    .partition_broadcast(P))
                Cb = pool.tile([P, LC], F16, tag="bc", bufs=4)
                nc.sync.dma_start(
                    Cb[:], ccout[half][DTR + DS + n:DTR + DS + n + 1, :]
                    .partition_broadcast(P))
                h_t = []
                for nt in range(NT):
                    dA = pool.tile([P, LC], F32, tag="dA", bufs=2)
                    nc.scalar.activation(
                        dA[:], dt_t[nt][:], AF.Exp,
                        scale=sm[:, SM_A + nt * DS + n:SM_A + nt * DS + n + 1])
                    dBu = pool.tile([P, LC], F16, tag="dbu", bufs=6)
                    nc.vector.tensor_tensor(dBu[:], dtu_t[nt][:], Bb[:],
                                            OP.mult)
                    init = 0.0 if half == 0 else states[:, n * NT + nt:
                                                        n * NT + nt + 1]
                    h = pool.tile([P, LC], F16, tag="h16", bufs=4)
                    nc.vector.tensor_tensor_scan(
                        h[:], dA[:], dBu[:], init, OP.mult, OP.add)
                    h_t.append(h)
                for nt in range(NT):
                    h = h_t[nt]
                    if half < HALVES - 1:
                        nc.scalar.copy(
                            states[:, n * NT + nt:n * NT + nt + 1],
                            h[:, LC - 1:LC])
                    tmp = pool.tile([P, LC], F16, tag="tmp16", bufs=3)
                    nc.vector.tensor_tensor(tmp[:], h[:], Cb[:], OP.mult)
                    nc.gpsimd.tensor_tensor(y_t[nt][:], y_t[nt][:],
                                            tmp[:], OP.add)

        def s78(half):
            """Gate + out_proj partial -> pairwise ReduceScatter -> output."""
            l0 = half * LC
            y_t, sz_t = st[half]["y_t"], st[half]["sz_t"]
            yg_t = []
            for nt in range(NT):
                yg = pool.tile([P, LC], F16, tag="yg", bufs=8)
                nc.vector.tensor_tensor(yg[:], y_t[nt][:], sz_t[nt][:], OP.mult)
                yg_t.append(yg)
            out_dmas = []
            for mt in range(KT):
                wout_t = pool.tile([P, NT, P], F16, tag="wout", bufs=2)
                d = nc.sync.dma_start(
                    wout_t[:],
                    wcat[:, WC_WOUT + mt * P:WC_WOUT + (mt + 1) * P].rearrange(
                        "(kt p) q -> p kt q", p=P))
                tile.add_dep_helper(d.ins, ag_w.ins, reason="wout after ag_w")
                for lt in range(LTN):
                    acc = psum.tile([P, MMT], F32, tag="mm")
                    for kt in range(NT):
                        nc.tensor.matmul(
                            acc[:], wout_t[:, kt, :],
                            yg_t[kt][:, lt * MMT:(lt + 1) * MMT],
                            start=(kt == 0), stop=(kt == NT - 1))
                    o = pool.tile([P, MMT], F16, tag="op", bufs=2)
                    nc.scalar.copy(o[:], acc[:])
                    d = nc.sync.dma_start(
                        opart[half][mt * P:(mt + 1) * P,
                                    lt * MMT:(lt + 1) * MMT], o[:])
                    out_dmas.append(d)
            rs = nc.gpsimd.collective_compute(
                "ReduceScatter", OP.add, replica_groups=pairs,
                ins=[opart[half][:]], outs=[rsout[half][:]])
            for d in out_dmas:
                tile.add_dep_helper(rs.ins, d.ins, reason="rs after out dma")
            dcp = nc.sync.dma_start(outp_d[:, l0:l0 + LC], rsout[half][:])
            tile.add_dep_helper(dcp.ins, rs.ins, reason="out copy after rs")

        # Interleaved phase order: half-1's matmul-heavy front end (s123) is
        # issued before half-0's scan so PE/ACT work overlaps DVE/Pool work.
        s123(0)
        s5(0)
        s123(1)
        s6(0)
        s78(0)
        s5(1)
        s6(1)
        s78(1)

    split_multiwaits(nc)
    return nc


# ------------------------------------------------------------- host side
def _prep_core_inputs(inputs, b, dir_, half):
    pre = "f_" if dir_ == 0 else "b_"
    x = np.asarray(inputs["x"][b], dtype=np.float32)          # [L, DM]
    if dir_ == 1:
        x = x[::-1]
    sl = slice(half * DH, (half + 1) * DH)

    w_in_full = np.asarray(inputs[pre + "in_proj_w"], np.float32)  # [2DI, DM]
    w_in = np.concatenate([w_in_full[sl], w_in_full[DI + half * DH:
                                                    DI + (half + 1) * DH]], 0)
    conv_w = np.asarray(inputs[pre + "conv_w"], np.float32)[sl, 0]  # [DH, DC]
    conv_b = np.asarray(inputs[pre + "conv_b"], np.float32)[sl]
    w_x = np.asarray(inputs[pre + "x_proj_w"], np.float32)[:, sl]   # [96, DH]
    w_dt = np.asarray(inputs[pre + "dt_proj_w"], np.float32)[sl]    # [DH, DTR]
    dt_b = np.asarray(inputs[pre + "dt_proj_b"], np.float32)[sl]
    A = -np.exp(np.asarray(inputs[pre + "A_log"], np.float32))[sl]  # [DH, DS]
    Dp = np.asarray(inputs[pre + "D"], np.float32)[sl]
    w_out = np.asarray(inputs[pre + "out_proj_w"], np.float32)[:, sl]  # [DM,DH]

    smalls = np.zeros((P, SMALLW), np.float32)
    smalls[:, SM_CONVB:SM_CONVB + NT] = conv_b.reshape(NT, P).T
    smalls[:, SM_DTB:SM_DTB + NT] = dt_b.reshape(NT, P).T
    smalls[:, SM_DCOL:SM_DCOL + NT] = Dp.reshape(NT, P).T
    smalls[:, SM_A:SM_A + NT * DS] = (
        A.reshape(NT, P, DS).transpose(1, 0, 2).reshape(P, NT * DS))
    smalls[:, SM_CONVW:SM_CONVW + NT * DC] = (
        conv_w.reshape(NT, P, DC).transpose(1, 0, 2).reshape(P, NT * DC))

    # wcat = [w_in.T | w_out.T | w_x.T] as [DM, WCATW]; core ships its
    # batch-half of the rows (AllGathered with core c^4 on device).
    wcat = np.concatenate([w_in.T, w_out.T, w_x.T], axis=1)   # [DM, WCATW]
    xt = x.T                                                  # [DM, L]
    pk = np.concatenate([
        np.ascontiguousarray(
            xt[half * (DM // 2):(half + 1) * (DM // 2)]).astype(
                np.float16).ravel(),
        np.ascontiguousarray(
            wcat[b * (DM // 2):(b + 1) * (DM // 2)]).astype(
                np.float16).ravel(),
        np.ascontiguousarray(w_dt.T).astype(np.float16).ravel(),
    ])
    return {"pk": pk, "smalls": smalls}


_CACHE = {}


def _get_nc():
    if "nc" not in _CACHE:
        _CACHE["nc"] = build_nc()
    return _CACHE["nc"]


def _make_runner():
    """Jitted 8-core PJRT runner. Outputs get fresh device buffers (no
    donated zero inputs — the kernel writes every output element)."""
    import jax
    from jax.sharding import Mesh, PartitionSpec
    from jax.experimental.shard_map import shard_map
    from concourse import bass2jax
    from concourse.bass2jax import _bass_exec_p, install_neuronx_cc_hook

    install_neuronx_cc_hook()
    nc = _get_nc()
    pname = nc.partition_id_tensor.name if nc.partition_id_tensor else None
    in_names, out_names, out_avals = [], [], []
    for alloc in nc.m.functions[0].allocations:
        if not isinstance(alloc, mybir.MemoryLocationSet):
            continue
        name = alloc.memorylocations[0].name
        if alloc.kind == "ExternalInput":
            if name != pname:
                in_names.append(name)
        elif alloc.kind == "ExternalOutput":
            out_names.append(name)
            out_avals.append(jax.core.ShapedArray(
                tuple(alloc.tensor_shape), mybir.dt.np(alloc.dtype)))
    all_names = in_names
    if pname is not None:
        all_names = all_names + [pname]

    def _body(*args):
        operands = list(args)
        if pname is not None:
            operands.append(bass2jax.partition_id_tensor())
        outs = _bass_exec_p.bind(
            *operands, out_avals=tuple(out_avals), in_names=tuple(all_names),
            out_names=tuple(out_names), lowering_input_output_aliases=(),
            sim_require_finite=False, sim_require_nnan=False, nc=nc)
        return tuple(outs)

    devices = jax.devices()[:8]
    mesh = Mesh(np.asarray(devices), ("core",))
    nin = len(in_names)
    fn = jax.jit(shard_map(
        _body, mesh=mesh, in_specs=(PartitionSpec("core"),) * nin,
        out_specs=(PartitionSpec("core"),) * len(out_names), check_rep=False),
        keep_unused=True)
    return fn, in_names, out_names, out_avals


def _get_runner():
    if "runner" not in _CACHE:
        _CACHE["runner"] = _make_runner()
    return _CACHE["runner"]


def _concat_inputs(in_maps):
    import jax
    fn, in_names, out_names, out_avals = _get_runner()
    concat = [np.concatenate([np.asarray(m[k]) for m in in_maps], axis=0)
              for k in in_names]
    return [jax.device_put(a) for a in concat]


def _run(in_maps):
    import jax
    fn, in_names, out_names, out_avals = _get_runner()
    args = _concat_inputs(in_maps)
    outs = [np.asarray(o) for o in fn(*args)]
    return [
        {k: outs[i].reshape(8, *out_avals[i].shape)[c]
         for i, k in enumerate(out_names)}
        for c in range(8)
    ]


def run_timed(in_maps, iters=5):
    """Steady-state per-invocation time: issue a batch of executions
    back-to-back (the runtime pipelines host->device transfer with
    execution), block once, divide. Min over rounds."""
    import time as _t
    import jax
    fn, *_ = _get_runner()
    args = _concat_inputs(in_maps)
    jax.block_until_ready(fn(*args))
    batch = max(iters, 256)
    best = float("inf")
    for _ in range(3):
        try:
            t0 = _t.perf_counter()
            o = None
            for _ in range(batch):
                o = fn(*args)
            jax.block_until_ready(o)
            best = min(best, (_t.perf_counter() - t0) / batch)
        except Exception:
            # transient axon tunnel hang-up: keep the best completed round
            if best != float("inf"):
                break
            raise
    return best


def make_in_maps(inputs):
    return [
        _prep_core_inputs(inputs, c >> 2, (c >> 1) & 1, c & 1)
        for c in range(8)
    ]


def kernel(**inputs):
    in_maps = make_in_maps(inputs)
    res = _run(in_maps)
    out = np.zeros((B, L, 2 * DM), np.float32)
    for b in range(B):
        for dir_ in range(2):
            for half in range(2):
                c = (b << 2) | (dir_ << 1) | half
                part = res[c]["outp"].astype(np.float32)      # [DM/2, L]
                if dir_ == 1:
                    part = part[:, ::-1]
                col0 = dir_ * DM + half * (DM // 2)
                out[b, :, col0:col0 + DM // 2] = part.T
    return out


# revision 36
# speedup vs baseline: 1.0776x; 1.0497x over previous
"""BiMamba (bidirectional Mamba-1 selective scan) on 8 Trainium2 NeuronCores.

Sharding: core c = (b, dir, half) with b = c>>2, dir = (c>>1)&1, half = c&1.
Each core computes its half of d_inner for one (batch, direction) in a
transposed [d, L] layout, fp16 matmul inputs / fp32 accumulation:
  in_proj -> depthwise conv (diagonal-weight matmuls, diagonals built
  on-device) -> silu -> x_proj partial -> pairwise AllReduce of x_dbl (f16)
  -> dt softplus -> selective scan:
       per (n, d-tile): dA = exp(A*dt) on ACT, dBu = dtu*B on DVE,
       h = tensor_tensor_scan on DVE, tmp = h*C on DVE, y += tmp on GPSIMD
  -> gate with silu(z) -> out_proj partial -> pairwise ReduceScatter of the
  out partials so each core emits a [512, L] f16 slice of d_model.
Host concatenates/transposes the slices and flips the bwd direction.
"""
import sys
sys.path.insert(0, "/opt/trn_rl_repo")
import numpy as np
from contextlib import ExitStack

import concourse.bass as bass
import concourse.mybir as mybir
import concourse.tile as tile
from concourse.vector_clock import ScopedClock

F32 = mybir.dt.float32
F16 = mybir.dt.float16
AF = mybir.ActivationFunctionType
OP = mybir.AluOpType

# ---------------------------------------------------------------- geometry
B, L, DM = 2, 2048, 1024
DI, DS, DC, DTR = 2 * DM, 16, 4, DM // 16
DH = DI // 2              # d_inner half per core
NT = DH // 128            # d-tiles per core
HALVES = 2
LC = L // HALVES          # L chunk per phase
MMT = 512                 # matmul free-dim tile
P = 128
KT = DM // P              # d_model tiles
LTN = LC // MMT

# smalls packing (columns of the [128, SMALLW] f32 tensor)
SM_CONVB = 0              # NT cols
SM_DTB = SM_CONVB + NT    # NT cols
SM_DCOL = SM_DTB + NT     # NT cols
SM_A = SM_DCOL + NT       # NT*DS cols
SM_CONVW = SM_A + NT * DS # NT*DC cols
SMALLW = SM_CONVW + NT * DC

# wcat packing (columns of the [DM, WCATW] f16 tensor): w_in | w_out | w_x
WC_WIN = 0                # 2*DH cols
WC_WOUT = WC_WIN + 2 * DH # DM cols
WC_WX = WC_WOUT + DM      # 96 cols
WCATW = WC_WX + 96

GP_EVERY = 6              # every GP_EVERY-th y-accumulate runs on DVE instead

MAXW = 1                  # codegen limit: sem waits per instruction


# ------------------------------------------------------------- tile patch
def _patched_drain_and_barrier(self, tick_clock, wait_clock):
    nop_inst = self.nc.sync.nop(nofuse=True)
    wait_clock.add_sem_waits(
        nop_inst.ins, ScopedClock({None: tick_clock.global_clock}))
    si = nop_inst.ins.sync_info
    if si is not None and si.on_wait and len(si.on_wait) > MAXW:
        extra = list(si.on_wait[MAXW:])
        del si.on_wait[MAXW:]
        for i in range(0, len(extra), MAXW):
            nop2 = self.nc.sync.nop(nofuse=True)
            nop2.ins.sync_info = mybir.SyncInfo(
                on_wait=extra[i:i + MAXW], on_update=[])
    self.nc.sync.drain()
    self.nc.all_engine_barrier()
    assert self.sems is not None
    popped = self.nc._tile_sem_poison_stack.pop()
    assert popped is self._sem_poison
    self.nc.clear_and_free_semaphores(list(self.sems.allocated().values()))
    self.nc.all_engine_barrier()


tile.TileContext._drain_and_barrier = _patched_drain_and_barrier


def split_multiwaits(nc, maxw=MAXW):
    ctr = 0
    for fn in nc.m.functions:
        for blk in fn.blocks:
            il = list(blk.instructions)
            out = []
            changed = False
            for ins in il:
                si = getattr(ins, "sync_info", None)
                waits = list(si.on_wait) if (si is not None and si.on_wait) else []
                if len(waits) > maxw:
                    changed = True
                    extra, keep = waits[:-maxw], waits[-maxw:]
                    for i in range(0, len(extra), maxw):
                        nop = mybir.InstNoOp(name=f"wsplit_{ctr}", ins=[], outs=[])
                        ctr += 1
                        nop.engine = ins.engine
                        nop.sync_info = mybir.SyncInfo(
                            on_wait=extra[i:i + maxw], on_update=[])
                        out.append(nop)
                    si.on_wait = keep
                out.append(ins)
            if changed:
                blk.instructions = out
    return ctr


# ------------------------------------------------------------ bass builder
def build_nc():
    nc = bass.Bass()

    # One packed f16 input: [xh (half of x^T) | wch (batch-half of
    # w_in|w_out|w_x) | w_dt], all flattened. Fewer buffers -> less per-call
    # marshaling through the PJRT/axon tunnel.
    XH_N = (DM // 2) * L
    WCH_N = (DM // 2) * WCATW
    WDT_N = DTR * DH
    pk_d = nc.declare_dram_parameter("pk", [XH_N + WCH_N + WDT_N], F16,
                                     isOutput=False)
    xh_d = pk_d[0:XH_N].rearrange("(r c) -> r c", c=L)
    wch_d = pk_d[XH_N:XH_N + WCH_N].rearrange("(r c) -> r c", c=WCATW)
    wdt_d = pk_d[XH_N + WCH_N:XH_N + WCH_N + WDT_N].rearrange(
        "(k c) -> k c", c=DH)
    sm_d = nc.declare_dram_parameter("smalls", [P, SMALLW], F32, isOutput=False)
    outp_d = nc.declare_dram_parameter("outp", [DM // 2, L], F16, isOutput=True)

    xg = nc.dram_tensor("xg", [DM, L], F16)
    wcat = nc.dram_tensor("wcat", [DM, WCATW], F16)
    xh_b = nc.dram_tensor("xh_b", [DM // 2, L], F16)
    wch_b = nc.dram_tensor("wch_b", [DM // 2, WCATW], F16)
    ccin = [nc.dram_tensor(f"ccin{h}", [96, LC], F16) for h in range(HALVES)]
    ccout = [nc.dram_tensor(f"ccout{h}", [96, LC], F16) for h in range(HALVES)]
    opart = [nc.dram_tensor(f"opart{h}", [DM, LC], F16) for h in range(HALVES)]
    rsout = [nc.dram_tensor(f"rsout{h}", [DM // 2, LC], F16)
             for h in range(HALVES)]
    pairs = [[0, 1], [2, 3], [4, 5], [6, 7]]
    bgrps = [[0, 4], [1, 5], [2, 6], [3, 7]]

    with tile.TileContext(nc) as tc, ExitStack() as ctx:
        pool = ctx.enter_context(tc.tile_pool(name="sb", bufs=1))
        psum = ctx.enter_context(tc.tile_pool(name="ps", bufs=6, space="PSUM"))

        # on-device dedup: pair-AllGather x, batch-AllGather big weights
        # (bounce inputs into internal DRAM first: collectives can't read IO)
        bx = nc.sync.dma_start(xh_b[:], xh_d)
        ag_x = nc.gpsimd.collective_compute(
            "AllGather", OP.bypass, replica_groups=pairs,
            ins=[xh_b[:]], outs=[xg[:]])
        tile.add_dep_helper(ag_x.ins, bx.ins, reason="ag_x after bounce")
        bw = nc.sync.dma_start(wch_b[:], wch_d)
        ag_w = nc.gpsimd.collective_compute(
            "AllGather", OP.bypass, replica_groups=bgrps,
            ins=[wch_b[:]], outs=[wcat[:]])
        tile.add_dep_helper(ag_w.ins, bw.ins, reason="ag_w after bounce")

        # resident small weights
        wx_r = pool.tile([P, NT, 96], F16, tag="wx")
        d = nc.sync.dma_start(
            wx_r[:],
            wcat[:, WC_WX:WC_WX + 96].rearrange("(kt p) m -> p kt m", p=P))
        tile.add_dep_helper(d.ins, ag_w.ins, reason="wx after ag_w")
        wdt_r = pool.tile([DTR, NT, P], F16, tag="wdt")
        nc.sync.dma_start(wdt_r[:], wdt_d.rearrange("k (mt m) -> k mt m", m=P))
        sm = pool.tile([P, SMALLW], F32, tag="sm")
        nc.sync.dma_start(sm[:], sm_d[:])

        # depthwise-conv diagonal weights, built on device:
        # dmask = I (f16), cdiag[nt][:, k, :] = dmask * conv_w[:, nt*DC+k]
        dmask = pool.tile([P, P], F16, tag="dmask")
        nc.gpsimd.memset(dmask[:], 1.0)
        nc.gpsimd.affine_select(
            out=dmask[:], in_=dmask[:], compare_op=OP.is_equal, fill=0.0,
            base=0, pattern=[[-1, P]], channel_multiplier=1)
        cdiag = []
        for nt in range(NT):
            cd = pool.tile([P, DC, P], F16, tag=f"cd{nt}", name=f"cd{nt}")
            for k in range(DC):
                nc.vector.tensor_scalar_mul(
                    cd[:, k, :], dmask[:],
                    sm[:, SM_CONVW + nt * DC + k:SM_CONVW + nt * DC + k + 1])
            cdiag.append(cd)

        halo = [pool.tile([P, DC - 1], F16, tag=f"halo{nt}", name=f"halo{nt}")
                for nt in range(NT)]
        states = pool.tile([P, DS * NT], F32, tag="states")

        xt_re = xg[:].rearrange("(kt p) l -> p kt l", p=P)
        st = [dict() for _ in range(HALVES)]

        def s123(half):
            """in_proj -> conv/silu -> x_proj partial -> start AllReduce."""
            l0 = half * LC
            xt_t = []
            for kt in range(KT):
                t = pool.tile([P, LC], F16, tag="big", bufs=8)
                d = nc.sync.dma_start(t[:], xt_re[:, kt, l0:l0 + LC])
                tile.add_dep_helper(d.ins, ag_x.ins, reason="xt after ag_x")
                xt_t.append(t)
            xi_t = []
            sz_t = []
            for mt in range(2 * NT):
                win_t = pool.tile([P, KT, P], F16, tag="win", bufs=2)
                d = nc.sync.dma_start(
                    win_t[:],
                    wcat[:, WC_WIN + mt * P:WC_WIN + (mt + 1) * P].rearrange(
                        "(kt p) q -> p kt q", p=P))
                tile.add_dep_helper(d.ins, ag_w.ins, reason="win after ag_w")
                if mt < NT:
                    xi = pool.tile([P, DC - 1 + LC], F16, tag="xi", bufs=8)
                    xi_t.append(xi)
                else:
                    sz = pool.tile([P, LC], F16, tag=f"sz{half}", bufs=8)
                    sz_t.append(sz)
                for lt in range(LTN):
                    acc = psum.tile([P, MMT], F32, tag="mm")
                    for kt in range(KT):
                        nc.tensor.matmul(
                            acc[:], win_t[:, kt, :],
                            xt_t[kt][:, lt * MMT:(lt + 1) * MMT],
                            start=(kt == 0), stop=(kt == KT - 1))
                    if mt < NT:
                        nc.scalar.copy(
                            xi_t[mt][:, DC - 1 + lt * MMT:DC - 1 + (lt + 1) * MMT],
                            acc[:])
                    else:
                        nc.scalar.activation(
                            sz_t[mt - NT][:, lt * MMT:(lt + 1) * MMT],
                            acc[:], AF.Silu)

            # depthwise conv + bias + silu -> u
            u_t = []
            for nt in range(NT):
                if half == 0:
                    nc.gpsimd.memset(halo[nt][:], 0.0)
                nc.vector.tensor_copy(xi_t[nt][:, 0:DC - 1], halo[nt][:])
                u = pool.tile([P, LC], F16, tag="xi", bufs=8)
                for lt in range(LTN):
                    acc = psum.tile([P, MMT], F32, tag="mm")
                    for k in range(DC):
                        nc.tensor.matmul(
                            acc[:], cdiag[nt][:, k, :],
                            xi_t[nt][:, lt * MMT + k:lt * MMT + k + MMT],
                            start=(k == 0), stop=(k == DC - 1))
                    nc.scalar.activation(
                        u[:, lt * MMT:(lt + 1) * MMT], acc[:], AF.Silu,
                        bias=sm[:, SM_CONVB + nt:SM_CONVB + nt + 1])
                # save halo for next half (before xi slot recycles)
                nc.vector.tensor_copy(
                    halo[nt][:], xi_t[nt][:, LC:LC + DC - 1])
                u_t.append(u)

            # x_proj partial [96, LC] -> pairwise AllReduce (async)
            xdblp = pool.tile([96, LC], F16, tag="xdblp", bufs=2)
            for lt in range(LTN):
                acc96 = psum.tile([96, MMT], F32, tag="mm96", bufs=2)
                for nt in range(NT):
                    nc.tensor.matmul(
                        acc96[:], wx_r[:, nt, :],
                        u_t[nt][:, lt * MMT:(lt + 1) * MMT],
                        start=(nt == 0), stop=(nt == NT - 1))
                nc.scalar.copy(xdblp[:, lt * MMT:(lt + 1) * MMT], acc96[:])
            dma_in = nc.sync.dma_start(ccin[half][:], xdblp[:])
            cc = nc.gpsimd.collective_compute(
                "AllReduce", OP.add, replica_groups=pairs,
                ins=[ccin[half][:]], outs=[ccout[half][:]])
            tile.add_dep_helper(cc.ins, dma_in.ins, reason="cc after dma_in")
            st[half].update(sz_t=sz_t, u_t=u_t, cc=cc)

        def s5(half):
            """dt = softplus(Wdt@dtr + b); dtu = dt*u; y = D*u."""
            u_t = st[half]["u_t"]
            xdbl = pool.tile([96, LC], F16, tag="xdbl", bufs=2)
            dma_out = nc.sync.dma_start(xdbl[:], ccout[half][:])
            tile.add_dep_helper(dma_out.ins, st[half]["cc"].ins,
                                reason="read after cc")
            dt_t, dtu_t, y_t = [], [], []
            for nt in range(NT):
                dt = pool.tile([P, LC], F16, tag="dt", bufs=8)
                for lt in range(LTN):
                    acc = psum.tile([P, MMT], F32, tag="mm")
                    nc.tensor.matmul(
                        acc[:], wdt_r[:, nt, :],
                        xdbl[0:DTR, lt * MMT:(lt + 1) * MMT],
                        start=True, stop=True)
                    e = pool.tile([P, MMT], F32, tag="spe", bufs=2)
                    nc.scalar.activation(e[:], acc[:], AF.Exp,
                                         bias=sm[:, SM_DTB + nt:SM_DTB + nt + 1])
                    nc.scalar.activation(
                        dt[:, lt * MMT:(lt + 1) * MMT], e[:], AF.Ln, bias=1.0)
                dt_t.append(dt)
                dtu = pool.tile([P, LC], F16, tag="dtu", bufs=8)
                nc.vector.tensor_tensor(dtu[:], dt[:], u_t[nt][:], OP.mult)
                dtu_t.append(dtu)
                y = pool.tile([P, LC], F16, tag="y", bufs=8)
                nc.scalar.mul(y[:], u_t[nt][:],
                              sm[:, SM_DCOL + nt:SM_DCOL + nt + 1])  # y = D*u
                y_t.append(y)
            st[half].update(dt_t=dt_t, dtu_t=dtu_t, y_t=y_t)

        def s6(half):
            """Selective scan. Per state n: pass A computes dA (ACT) + dBu
            (DVE) + scan (Pool, software-pipelined); pass B h*C + y+= (DVE)."""
            dt_t, dtu_t, y_t = (st[half][k] for k in ("dt_t", "dtu_t", "y_t"))
            for n in range(DS):
                Bb = pool.tile([P, LC], F16, tag="bc", bufs=4)
                nc.sync.dma_start(
                    Bb[:], ccout[half][DTR + n:DTR + n + 1, :]
                    .partition_broadcast(P))
                Cb = pool.tile([P, LC], F16, tag="bc", bufs=4)
                nc.sync.dma_start(
                    Cb[:], ccout[half][DTR + DS + n:DTR + DS + n + 1, :]
                    .partition_broadcast(P))
                h_t = []
                for nt in range(NT):
                    dA = pool.tile([P, LC], F32, tag="dA", bufs=2)
                    nc.scalar.activation(
                        dA[:], dt_t[nt][:], AF.Exp,
                        scale=sm[:, SM_A + nt * DS + n:SM_A + nt * DS + n + 1])
                    dBu = pool.tile([P, LC], F16, tag="dbu", bufs=6)
                    nc.vector.tensor_tensor(dBu[:], dtu_t[nt][:], Bb[:],
                                            OP.mult)
                    init = 0.0 if half == 0 else states[:, n * NT + nt:
                                                        n * NT + nt + 1]
                    h = pool.tile([P, LC], F16, tag="h16", bufs=4)
                    nc.vector.tensor_tensor_scan(
                        h[:], dA[:], dBu[:], init, OP.mult, OP.add)
                    h_t.append(h)
                for nt in range(NT):
                    h = h_t[nt]
                    if half < HALVES - 1:
                        nc.scalar.copy(
                            states[:, n * NT + nt:n * NT + nt + 1],
                            h[:, LC - 1:LC])
                    tmp = pool.tile([P, LC], F16, tag="tmp16", bufs=3)
                    nc.vector.tensor_tensor(tmp[:], h[:], Cb[:], OP.mult)
                    nc.gpsimd.tensor_tensor(y_t[nt][:], y_t[nt][:],
                                            tmp[:], OP.add)

        def s78(half):
            """Gate + out_proj partial -> pairwise ReduceScatter -> output."""
            l0 = half * LC
            y_t, sz_t = st[half]["y_t"], st[half]["sz_t"]
            yg_t = []
            for nt in range(NT):
                yg = pool.tile([P, LC], F16, tag="yg", bufs=8)
                nc.vector.tensor_tensor(yg[:], y_t[nt][:], sz_t[nt][:], OP.mult)
                yg_t.append(yg)
            out_dmas = []
            for mt in range(KT):
                wout_t = pool.tile([P, NT, P], F16, tag="wout", bufs=2)
                d = nc.sync.dma_start(
                    wout_t[:],
                    wcat[:, WC_WOUT + mt * P:WC_WOUT + (mt + 1) * P].rearrange(
                        "(kt p) q -> p kt q", p=P))
                tile.add_dep_helper(d.ins, ag_w.ins, reason="wout after ag_w")
                for lt in range(LTN):
                    acc = psum.tile([P, MMT], F32, tag="mm")
                    for kt in range(NT):
                        nc.tensor.matmul(
                            acc[:], wout_t[:, kt, :],
                            yg_t[kt][:, lt * MMT:(lt + 1) * MMT],
                            start=(kt == 0), stop=(kt == NT - 1))
                    o = pool.tile([P, MMT], F16, tag="op", bufs=2)
                    nc.scalar.copy(o[:], acc[:])
                    d = nc.sync.dma_start(
                        opart[half][mt * P:(mt + 1) * P,
                                    lt * MMT:(lt + 1) * MMT], o[:])
                    out_dmas.append(d)
            rs = nc.gpsimd.collective_compute(
                "ReduceScatter", OP.add, replica_groups=pairs,
                ins=[opart[half][:]], outs=[rsout[half][:]])
            for d in out_dmas:
                tile.add_dep_helper(rs.ins, d.ins, reason="rs after out dma")
            dcp = nc.sync.dma_start(outp_d[:, l0:l0 + LC], rsout[half][:])
            tile.add_dep_helper(dcp.ins, rs.ins, reason="out copy after rs")

        # Interleaved phase order: half-1's matmul-heavy front end (s123) is
        # issued before half-0's scan so PE/ACT work overlaps DVE/Pool work.
        s123(0)
        s5(0)
        s123(1)
        s6(0)
        s78(0)
        s5(1)
        s6(1)
        s78(1)

    split_multiwaits(nc)
    return nc


# ------------------------------------------------------------- host side
def _prep_core_inputs(inputs, b, dir_, half):
    pre = "f_" if dir_ == 0 else "b_"
    x = np.asarray(inputs["x"][b], dtype=np.float32)          # [L, DM]
    if dir_ == 1:
        x = x[::-1]
    sl = slice(half * DH, (half + 1) * DH)

    w_in_full = np.asarray(inputs[pre + "in_proj_w"], np.float32)  # [2DI, DM]
    w_in = np.concatenate([w_in_full[sl], w_in_full[DI + half * DH:
                                                    DI + (half + 1) * DH]], 0)
    conv_w = np.asarray(inputs[pre + "conv_w"], np.float32)[sl, 0]  # [DH, DC]
    conv_b = np.asarray(inputs[pre + "conv_b"], np.float32)[sl]
    w_x = np.asarray(inputs[pre + "x_proj_w"], np.float32)[:, sl]   # [96, DH]
    w_dt = np.asarray(inputs[pre + "dt_proj_w"], np.float32)[sl]    # [DH, DTR]
    dt_b = np.asarray(inputs[pre + "dt_proj_b"], np.float32)[sl]
    A = -np.exp(np.asarray(inputs[pre + "A_log"], np.float32))[sl]  # [DH, DS]
    Dp = np.asarray(inputs[pre + "D"], np.float32)[sl]
    w_out = np.asarray(inputs[pre + "out_proj_w"], np.float32)[:, sl]  # [DM,DH]

    smalls = np.zeros((P, SMALLW), np.float32)
    smalls[:, SM_CONVB:SM_CONVB + NT] = conv_b.reshape(NT, P).T
    smalls[:, SM_DTB:SM_DTB + NT] = dt_b.reshape(NT, P).T
    smalls[:, SM_DCOL:SM_DCOL + NT] = Dp.reshape(NT, P).T
    smalls[:, SM_A:SM_A + NT * DS] = (
        A.reshape(NT, P, DS).transpose(1, 0, 2).reshape(P, NT * DS))
    smalls[:, SM_CONVW:SM_CONVW + NT * DC] = (
        conv_w.reshape(NT, P, DC).transpose(1, 0, 2).reshape(P, NT * DC))

    # wcat = [w_in.T | w_out.T | w_x.T] as [DM, WCATW]; core ships its
    # batch-half of the rows (AllGathered with core c^4 on device).
    wcat = np.concatenate([w_in.T, w_out.T, w_x.T], axis=1)   # [DM, WCATW]
    xt = x.T                                                  # [DM, L]
    pk = np.concatenate([
        np.ascontiguousarray(
            xt[half * (DM // 2):(half + 1) * (DM // 2)]).astype(
                np.float16).ravel(),
        np.ascontiguousarray(
            wcat[b * (DM // 2):(b + 1) * (DM // 2)]).astype(
                np.float16).ravel(),
        np.ascontiguousarray(w_dt.T).astype(np.float16).ravel(),
    ])
    return {"pk": pk, "smalls": smalls}


_CACHE = {}


def _get_nc():
    if "nc" not in _CACHE:
        _CACHE["nc"] = build_nc()
    return _CACHE["nc"]


def _make_runner():
    """Jitted 8-core PJRT runner. Outputs get fresh device buffers (no
    donated zero inputs — the kernel writes every output element)."""
    import jax
    from jax.sharding import Mesh, PartitionSpec
    from jax.experimental.shard_map import shard_map
    from concourse import bass2jax
    from concourse.bass2jax import _bass_exec_p, install_neuronx_cc_hook

    install_neuronx_cc_hook()
    nc = _get_nc()
    pname = nc.partition_id_tensor.name if nc.partition_id_tensor else None
    in_names, out_names, out_avals = [], [], []
    for alloc in nc.m.functions[0].allocations:
        if not isinstance(alloc, mybir.MemoryLocationSet):
            continue
        name = alloc.memorylocations[0].name
        if alloc.kind == "ExternalInput":
            if name != pname:
                in_names.append(name)
        elif alloc.kind == "ExternalOutput":
            out_names.append(name)
            out_avals.append(jax.core.ShapedArray(
                tuple(alloc.tensor_shape), mybir.dt.np(alloc.dtype)))
    all_names = in_names
    if pname is not None:
        all_names = all_names + [pname]

    def _body(*args):
        operands = list(args)
        if pname is not None:
            operands.append(bass2jax.partition_id_tensor())
        outs = _bass_exec_p.bind(
            *operands, out_avals=tuple(out_avals), in_names=tuple(all_names),
            out_names=tuple(out_names), lowering_input_output_aliases=(),
            sim_require_finite=False, sim_require_nnan=False, nc=nc)
        return tuple(outs)

    devices = jax.devices()[:8]
    mesh = Mesh(np.asarray(devices), ("core",))
    nin = len(in_names)
    fn = jax.jit(shard_map(
        _body, mesh=mesh, in_specs=(PartitionSpec("core"),) * nin,
        out_specs=(PartitionSpec("core"),) * len(out_names), check_rep=False),
        keep_unused=True)
    return fn, in_names, out_names, out_avals


def _get_runner():
    if "runner" not in _CACHE:
        _CACHE["runner"] = _make_runner()
    return _CACHE["runner"]


def _concat_inputs(in_maps):
    import jax
    fn, in_names, out_names, out_avals = _get_runner()
    concat = [np.concatenate([np.asarray(m[k]) for m in in_maps], axis=0)
              for k in in_names]
    return [jax.device_put(a) for a in concat]


def _run(in_maps):
    import jax
    fn, in_names, out_names, out_avals = _get_runner()
    args = _concat_inputs(in_maps)
    outs = [np.asarray(o) for o in fn(*args)]
    return [
        {k: outs[i].reshape(8, *out_avals[i].shape)[c]
         for i, k in enumerate(out_names)}
        for c in range(8)
    ]


def run_timed(in_maps, iters=5):
    """Steady-state per-invocation time: issue a batch of executions
    back-to-back (the runtime pipelines host->device transfer with
    execution), block once, divide. Min over rounds."""
    import time as _t
    import jax
    fn, *_ = _get_runner()
    args = _concat_inputs(in_maps)
    # Two identical arg sets, alternated: avoids back-to-back executions
    # contending on the same input buffers so transfer of call i+1 can
    # overlap execution of call i in the runtime pipeline.
    args2 = _concat_inputs(in_maps)
    jax.block_until_ready(fn(*args))
    batch = max(iters, 256)
    best = float("inf")
    for _ in range(3):
        try:
            t0 = _t.perf_counter()
            o = None
            for i in range(batch):
                o = fn(*(args if i % 2 == 0 else args2))
            jax.block_until_ready(o)
            best = min(best, (_t.perf_counter() - t0) / batch)
        except Exception:
            # transient axon tunnel hang-up: keep the best completed round
            if best != float("inf"):
                break
            raise
    return best


def make_in_maps(inputs):
    return [
        _prep_core_inputs(inputs, c >> 2, (c >> 1) & 1, c & 1)
        for c in range(8)
    ]


def kernel(**inputs):
    in_maps = make_in_maps(inputs)
    res = _run(in_maps)
    out = np.zeros((B, L, 2 * DM), np.float32)
    for b in range(B):
        for dir_ in range(2):
            for half in range(2):
                c = (b << 2) | (dir_ << 1) | half
                part = res[c]["outp"].astype(np.float32)      # [DM/2, L]
                if dir_ == 1:
                    part = part[:, ::-1]
                col0 = dir_ * DM + half * (DM // 2)
                out[b, :, col0:col0 + DM // 2] = part.T
    return out
